# revision 1
# baseline (speedup 1.0000x reference)
"""Expert-parallel MoE MLP (top-2 of 8 experts) on 8 TRN2 NeuronCores.

Strategy (expert-parallel, per sharding hint):
  - core e holds expert e's weights (w1[e], w2[e], host-pre-transposed, bf16)
  - host dispatches tokens by expert id: core e receives the tokens routed to
    expert e in a block-padded layout split into sub-buffers (see _sub_bounds);
    sub-buffer g holds one slot-range of every (expert->owner) block, so the
    AllToAll splits into chunks that fire as compute progresses (symmetric
    halves measured best: 3 asymmetric chunks paid more in entry floors than
    the earlier firing saved)
  - core e computes y_part = [silu(x_e @ w1[e]^T) * c_e] @ w2[e]^T for its
    tokens (bf16 matmuls, fp32 accumulate, combine weights applied in fp32,
    partials exchanged in bf16)
  - G chunked AllToAlls return each owner core the partial rows for its 512
    tokens; the owner gathers the (up to) 2 partial rows per token, adds
    them, and writes its [512, 1024] fp32 output shard
  - host concatenates the 8 output shards
"""

import sys

sys.path.insert(0, "/opt/trn_rl_repo")

import numpy as np
import ml_dtypes

import concourse.bass as bass
import concourse.tile as tile
from concourse import bacc, mybir
from concourse.bass_utils import run_bass_kernel_spmd

S, DM, DF, E, TOPK = 4096, 1024, 2048, 8, 2
NCORES = 8
P = 128
OWN = S // NCORES  # tokens per owner core
G = 2  # number of chunked AllToAlls
MM1_STATIONARY = True  # reuse w1 as stationary across chunks (1 ldw : n_chunk mms)

_PROGRAM_CACHE: dict = {}


def _sub_bounds(blk: int) -> list[int]:
    """Slot-range boundaries for the chunked A2As. The split lands on an
    m-tile row boundary (8*b1 % 128 == 0 when b1 % 16 == 0) just past the
    midpoint: chunk 0 fires at the same compute tile as a 50/50 split would,
    but the exposed final chunk is ~10% smaller."""
    b1 = max(16, (blk * 5 // 9) // 16 * 16)
    return [0, b1, blk] if 0 < b1 < blk else [0, blk]


def _chunks_of(tokpad: int) -> list[tuple[int, int]]:
    """Split tokpad into (start, size) chunks, each a multiple of 128, <= 512."""
    n_ch = -(-tokpad // 512)
    base = tokpad // n_ch // P * P
    sizes = [base] * n_ch
    rem = tokpad - base * n_ch
    i = 0
    while rem > 0:
        sizes[i] += P
        rem -= P
        i = (i + 1) % n_ch
    out, pos = [], 0
    for s in sizes:
        out.append((pos, s))
        pos += s
    assert pos == tokpad
    return out


def _emit(nc, tc, ctx, tokpad: int, reps: int, mode: str = "full"):
    dt = mybir.dt
    ntokm = tokpad // P
    chunks = _chunks_of(tokpad)
    bounds = _sub_bounds(tokpad // NCORES)  # slot boundaries per block
    row_bounds = [NCORES * b for b in bounds]  # sendbuf row boundaries
    n_a2a = len(bounds) - 1

    xT = nc.dram_tensor("xT", [DM, tokpad], dt.bfloat16, kind="ExternalInput").ap()
    w1t = nc.dram_tensor("w1t", [DM, DF], dt.bfloat16, kind="ExternalInput").ap()
    w2t = nc.dram_tensor("w2t", [DF, DM], dt.bfloat16, kind="ExternalInput").ap()
    cv = nc.dram_tensor("cv", [tokpad], dt.float32, kind="ExternalInput").ap()
    g0 = nc.dram_tensor("g0", [OWN], dt.int32, kind="ExternalInput").ap()
    g1 = nc.dram_tensor("g1", [OWN], dt.int32, kind="ExternalInput").ap()
    yout = nc.dram_tensor("yout", [OWN, DM], dt.float32, kind="ExternalOutput").ap()
    sendbuf = nc.dram_tensor("sendbuf", [tokpad, DM], dt.bfloat16).ap()
    recvbuf = nc.dram_tensor("recvbuf", [tokpad + 1, DM], dt.bfloat16).ap()

    n_chunks = len(chunks)
    wpool = ctx.enter_context(tc.tile_pool(name="w", bufs=1))
    hpool = ctx.enter_context(
        tc.tile_pool(name="h", bufs=(DF // P) * n_chunks + 2 if MM1_STATIONARY else 20)
    )
    ypool = ctx.enter_context(tc.tile_pool(name="y", bufs=4))
    gpool = ctx.enter_context(tc.tile_pool(name="g", bufs=4))
    phpool = ctx.enter_context(
        tc.tile_pool(name="ph", bufs=n_chunks + 1 if MM1_STATIONARY else 2, space="PSUM")
    )
    pypool = ctx.enter_context(tc.tile_pool(name="py", bufs=4, space="PSUM"))

    for _rep in range(reps):
        # ---- loads: w1/x interleaved (first matmuls need them), then the rest
        w1sb = wpool.tile([P, DM // P, DF], dt.bfloat16, tag="w1sb")
        w1r = w1t.rearrange("(o p) f -> p o f", p=P)
        xsb = wpool.tile([P, DM // P, tokpad], dt.bfloat16, tag="xsb")
        xr = xT.rearrange("(o p) t -> p o t", p=P)
        for k in range(DM // P):
            nc.sync.dma_start(w1sb[:, k, :], w1r[:, k, :])
            nc.sync.dma_start(xsb[:, k, :], xr[:, k, :])
        csb = wpool.tile([P, ntokm], dt.float32, tag="csb")
        nc.sync.dma_start(csb[:], cv.rearrange("(t p) -> p t", p=P))
        w2sb = wpool.tile([P, DF // P, DM], dt.bfloat16, tag="w2sb")
        w2r = w2t.rearrange("(o p) d -> p o d", p=P)
        for f in range(DF // P):
            nc.sync.dma_start(w2sb[:, f, :], w2r[:, f, :])
        g0sb = wpool.tile([P, OWN // P], dt.int32, tag="g0sb")
        nc.sync.dma_start(g0sb[:], g0.rearrange("(t p) -> p t", p=P))
        g1sb = wpool.tile([P, OWN // P], dt.int32, tag="g1sb")
        nc.sync.dma_start(g1sb[:], g1.rearrange("(t p) -> p t", p=P))
        zrow = wpool.tile([1, DM], dt.bfloat16, tag="zrow")
        nc.vector.memset(zrow[:], 0.0)
        nc.sync.dma_start(recvbuf[tokpad : tokpad + 1, :], zrow[:])

        # ---- expert MLP over token chunks; fire A2A chunk g once its rows exist
        a2a_fired = 0

        def fire_a2a(upto_row):
            nonlocal a2a_fired
            while (
                mode != "compute"
                and a2a_fired < n_a2a
                and row_bounds[a2a_fired + 1] <= upto_row
            ):
                r0, r1 = row_bounds[a2a_fired], row_bounds[a2a_fired + 1]
                nc.gpsimd.collective_compute(
                    "AllToAll",
                    mybir.AluOpType.bypass,
                    replica_groups=[list(range(NCORES))],
                    ins=[sendbuf[r0:r1, :]],
                    outs=[recvbuf[r0:r1, :]],
                )
                a2a_fired += 1

        def mm2_tile(hs_j, c0, csz, tm):  # token m-tile: y = (c * H^T) . w2
            tglob = c0 // P + tm
            py0 = pypool.tile([P, 512], dt.float32, tag="py")
            py1 = pypool.tile([P, 512], dt.float32, tag="py")
            for f in range(DF // P):
                lhs = hs_j[f][:, tm * P : (tm + 1) * P]
                nc.tensor.matmul(
                    py0[:], lhsT=lhs, rhs=w2sb[:, f, 0:512],
                    start=(f == 0), stop=(f == DF // P - 1),
                )
                nc.tensor.matmul(
                    py1[:], lhsT=lhs, rhs=w2sb[:, f, 512:1024],
                    start=(f == 0), stop=(f == DF // P - 1),
                )
            y_sb = ypool.tile([P, DM], dt.bfloat16, tag="y")
            nc.vector.tensor_scalar_mul(
                y_sb[:, 0:512], py0[:], csb[:, tglob : tglob + 1]
            )
            nc.vector.tensor_scalar_mul(
                y_sb[:, 512:1024], py1[:], csb[:, tglob : tglob + 1]
            )
            nc.sync.dma_start(sendbuf[tglob * P : (tglob + 1) * P, :], y_sb[:])
            fire_a2a((tglob + 1) * P)

        def silu_to_h(ph, csz):
            h_i = hpool.tile([P, csz], dt.bfloat16, tag="h")
            nc.scalar.activation(h_i[:], ph[:], mybir.ActivationFunctionType.Silu)
            return h_i

        if mode != "a2a":
            if MM1_STATIONARY:
                hs_all = [[None] * (DF // P) for _ in chunks]
                for i in range(DF // P):
                    phs = [
                        phpool.tile([P, csz], dt.float32, tag="ph", name=f"ph{j}")
                        for j, (_c0, csz) in enumerate(chunks)
                    ]
                    for k in range(DM // P):
                        for j, (c0, csz) in enumerate(chunks):
                            nc.tensor.matmul(
                                phs[j][:],
                                lhsT=w1sb[:, k, i * P : (i + 1) * P],
                                rhs=xsb[:, k, c0 : c0 + csz],
                                start=(k == 0),
                                stop=(k == DM // P - 1),
                            )
                    for j, (_c0, csz) in enumerate(chunks):
                        hs_all[j][i] = silu_to_h(phs[j], csz)
                for j, (c0, csz) in enumerate(chunks):
                    for tm in range(csz // P):
                        mm2_tile(hs_all[j], c0, csz, tm)
            else:
                for c0, csz in chunks:
                    hs = []
                    for i in range(DF // P):  # f-tiles: H[f] = silu(w1 . x)
                        ph = phpool.tile([P, csz], dt.float32, tag="ph")
                        for k in range(DM // P):
                            nc.tensor.matmul(
                                ph[:],
                                lhsT=w1sb[:, k, i * P : (i + 1) * P],
                                rhs=xsb[:, k, c0 : c0 + csz],
                                start=(k == 0),
                                stop=(k == DM // P - 1),
                            )
                        hs.append(silu_to_h(ph, csz))
                    for tm in range(csz // P):
                        mm2_tile(hs, c0, csz, tm)
            fire_a2a(tokpad)
        else:
            fire_a2a(tokpad)

        # ---- combine: per owned token, add its (up to) 2 partial rows ----
        for tm in range(OWN // P):
            ga = gpool.tile([P, DM], dt.bfloat16, tag="ga")
            nc.gpsimd.indirect_dma_start(
                out=ga[:],
                out_offset=None,
                in_=recvbuf[:],
                in_offset=bass.IndirectOffsetOnAxis(ap=g0sb[:, tm : tm + 1], axis=0),
            )
            gb = gpool.tile([P, DM], dt.bfloat16, tag="gb")
            nc.gpsimd.indirect_dma_start(
                out=gb[:],
                out_offset=None,
                in_=recvbuf[:],
                in_offset=bass.IndirectOffsetOnAxis(ap=g1sb[:, tm : tm + 1], axis=0),
            )
            ys = gpool.tile([P, DM], dt.float32, tag="ys")
            nc.vector.tensor_add(ys[:], ga[:], gb[:])
            nc.sync.dma_start(yout[tm * P : (tm + 1) * P, :], ys[:])


def _build_program(tokpad: int, reps: int = 1, mode: str = "full"):
    key = (tokpad, reps, mode, "v4", MM1_STATIONARY)
    if key in _PROGRAM_CACHE:
        return _PROGRAM_CACHE[key]
    from contextlib import ExitStack

    nc = bacc.Bacc(
        "TRN2",
        target_bir_lowering=False,
        debug=False,
        enable_asserts=True,
        num_devices=NCORES,
    )
    with tile.TileContext(nc) as tc:
        with ExitStack() as ctx:
            _emit(nc, tc, ctx, tokpad, reps, mode)
    nc.compile()
    _PROGRAM_CACHE[key] = nc
    return nc


def _prepare(x, topk_e, topk_w):
    """Host-side routing: dispatch tokens to experts.

    Column/sendbuf-row layout on core e (tokpad = 8*BLK rows, G sub-buffers):
      token with slot s in (expert e -> owner d) block lives at row
        (s // SUBBLK) * (8*SUBBLK) + d * SUBBLK + (s % SUBBLK)
    so rows [g*tokpad/G, (g+1)*tokpad/G) form A2A chunk g = slot-range
    [g*SUBBLK, (g+1)*SUBBLK) of all 8 destination blocks.
    """
    bf16 = ml_dtypes.bfloat16
    c = np.zeros((S, E), dtype=np.float32)
    np.add.at(c, (np.arange(S)[:, None], topk_e), topk_w.astype(np.float32))

    toks = [np.nonzero((topk_e == e).any(axis=1))[0] for e in range(E)]
    cnt = np.zeros((E, NCORES), dtype=np.int64)
    for e in range(E):
        d = toks[e] // OWN
        for dd in range(NCORES):
            cnt[e, dd] = int((d == dd).sum())
    blk = int(-(-cnt.max() // 16) * 16)  # multiple of 16 (so tokpad % 128 == 0)
    tokpad = blk * NCORES
    zero_row = tokpad
    bounds = _sub_bounds(blk)

    def row_of(d, s):
        for g in range(len(bounds) - 1):
            if s < bounds[g + 1]:
                sz = bounds[g + 1] - bounds[g]
                return NCORES * bounds[g] + d * sz + (s - bounds[g])
        raise AssertionError(s)

    in_maps = []
    slot_of = {}
    for e in range(E):
        te = toks[e]
        d = te // OWN
        seg_start = np.searchsorted(te, np.arange(NCORES) * OWN)
        slots = np.arange(len(te)) - seg_start[d]
        col = np.array([row_of(dd, ss) for dd, ss in zip(d, slots)], dtype=np.int64)
        for t, sl in zip(te, slots):
            slot_of[(e, int(t))] = int(sl)
        xT_e = np.zeros((DM, tokpad), dtype=bf16)
        if len(te):
            xT_e[:, col] = x[te].T.astype(bf16)
        cv_e = np.zeros(tokpad, dtype=np.float32)
        cv_e[col] = c[te, e]
        in_maps.append({"xT": xT_e, "cv": cv_e})

    for d in range(NCORES):
        g0a = np.full(OWN, zero_row, dtype=np.int32)
        g1a = np.full(OWN, zero_row, dtype=np.int32)
        for t_loc in range(OWN):
            t = d * OWN + t_loc
            es = np.unique(topk_e[t])
            g0a[t_loc] = row_of(int(es[0]), slot_of[(int(es[0]), t)])
            if len(es) > 1:
                g1a[t_loc] = row_of(int(es[1]), slot_of[(int(es[1]), t)])
        in_maps[d]["g0"] = g0a
        in_maps[d]["g1"] = g1a

    return in_maps, tokpad


def prepare_in_maps(x, topk_e, topk_w, w1, w2):
    bf16 = ml_dtypes.bfloat16
    in_maps, tokpad = _prepare(
        np.asarray(x), np.asarray(topk_e), np.asarray(topk_w)
    )
    for e in range(E):
        in_maps[e]["w1t"] = np.ascontiguousarray(np.asarray(w1)[e].T).astype(bf16)
        in_maps[e]["w2t"] = np.ascontiguousarray(np.asarray(w2)[e].T).astype(bf16)
    return in_maps, tokpad


def kernel(x, topk_e, topk_w, w1, w2):
    in_maps, tokpad = prepare_in_maps(x, topk_e, topk_w, w1, w2)
    nc = _build_program(tokpad)
    res = run_bass_kernel_spmd(nc, in_maps, list(range(NCORES)))
    out = np.concatenate(
        [res.results[d]["yout"] for d in range(NCORES)], axis=0
    )
    return out.astype(np.float32)



# revision 2
# speedup vs baseline: 94689.1670x; 94689.1670x over previous
"""Expert-parallel MoE MLP (top-2 of 8 experts) on 8 TRN2 NeuronCores.

Strategy (expert-parallel, per sharding hint):
  - core e holds expert e's weights (w1[e], w2[e], host-pre-transposed, bf16)
  - host dispatches tokens by expert id into a COMPACT layout: core e's
    xT holds exactly its routed tokens (ordered by owned-segment, then
    owner, then token id), padded only at the tail to a multiple of 128.
    This minimizes matmul columns (vs block-padded dispatch).
  - core e computes y = [silu(x_e @ w1[e]^T) * c_e] @ w2[e]^T over
    512-column chunks (bf16 matmuls, fp32 accumulate).
  - mm2 output m-tiles are indirect-scattered into `sendbuf`, which holds
    the AllToAll wire layout: NSEG slot-segments, stored in REVERSE
    segment order (plus a leading trash region for pad rows) so that each
    scatter's AP byte-extent is a prefix that never overlaps the A2A read
    slices of earlier-fired segments (avoids false WAR serialization).
  - NSEG chunked AllToAlls fire as soon as the last m-tile holding each
    segment's tokens has been scattered; segment s holds the partial rows
    of owned-token m-tile s on every owner, so the owner-side combine
    (2 indirect row-gathers + add) runs right after each A2A lands,
    overlapped with remaining compute and later A2As.
  - 16 dummy matmuls at t=0 warm the PE HAM clock gate during the loads.
"""

import sys

sys.path.insert(0, "/opt/trn_rl_repo")

import numpy as np
import ml_dtypes

import concourse.bass as bass
import concourse.tile as tile
from concourse import bacc, mybir
from concourse.bass_utils import run_bass_kernel_spmd

S, DM, DF, E, TOPK = 4096, 1024, 2048, 8, 2
NCORES = 8
P = 128
OWN = S // NCORES  # tokens per owner core
OWNSEG = 128  # owned tokens per A2A segment (= one combine m-tile)
NSEG = OWN // OWNSEG

_PROGRAM_CACHE: dict = {}


def _chunks_of(ntok: int) -> list[tuple[int, int]]:
    """Split ntok into (start, size) chunks, each a multiple of 128, <= 512."""
    out, pos = [], 0
    while pos < ntok:
        sz = min(512, ntok - pos)
        out.append((pos, sz))
        pos += sz
    return out


def _emit(nc, tc, ctx, plan: dict):
    dt = mybir.dt
    ntok = plan["ntok"]
    SS = plan["SS"]  # per-seg slot-block size
    RS = plan["RS"]  # per-seg row start in sendbuf (reverse layout)
    FI = plan["FI"]  # fire A2A-s after scatter of m-tile FI[s]
    BND = plan["BND"]  # per-m-tile scatter AP row bound (prefix extent)
    nmt = ntok // P
    blk = sum(SS)
    sendrows = P + 8 * blk  # leading P trash rows + segments

    xT = nc.dram_tensor("xT", [DM, ntok], dt.bfloat16, kind="ExternalInput").ap()
    w1t = nc.dram_tensor("w1t", [DM, DF], dt.bfloat16, kind="ExternalInput").ap()
    w2t = nc.dram_tensor("w2t", [DF, DM], dt.bfloat16, kind="ExternalInput").ap()
    cv = nc.dram_tensor("cv", [ntok], dt.float32, kind="ExternalInput").ap()
    scat = nc.dram_tensor("scat", [ntok], dt.int32, kind="ExternalInput").ap()
    g0 = nc.dram_tensor("g0", [OWN], dt.int32, kind="ExternalInput").ap()
    g1 = nc.dram_tensor("g1", [OWN], dt.int32, kind="ExternalInput").ap()
    yout = nc.dram_tensor("yout", [OWN, DM], dt.float32, kind="ExternalOutput").ap()
    sendbuf = nc.dram_tensor("sendbuf", [sendrows, DM], dt.bfloat16).ap()
    recv = [
        nc.dram_tensor(f"recv{s}", [8 * SS[s] + 1, DM], dt.bfloat16).ap()
        for s in range(NSEG)
    ]

    wpool = ctx.enter_context(tc.tile_pool(name="w", bufs=1))
    hpool = ctx.enter_context(tc.tile_pool(name="h", bufs=34))
    ypool = ctx.enter_context(tc.tile_pool(name="y", bufs=6))
    gpool = ctx.enter_context(tc.tile_pool(name="g", bufs=2))
    phpool = ctx.enter_context(tc.tile_pool(name="ph", bufs=2, space="PSUM"))
    pypool = ctx.enter_context(tc.tile_pool(name="py", bufs=4, space="PSUM"))
    pwpool = ctx.enter_context(tc.tile_pool(name="pw", bufs=1, space="PSUM"))

    # ---- PE warmup: release the HAM clock gate while DMAs load ----
    warm = wpool.tile([P, 512], dt.bfloat16, tag="warm")
    nc.vector.memset(warm[:], 0.0)
    pw = pwpool.tile([P, 512], dt.float32, tag="pw")
    for _ in range(16):
        nc.tensor.matmul(pw[:], lhsT=warm[:, 0:P], rhs=warm[:], start=True, stop=True)

    # ---- loads: w1/x interleaved (first matmuls need them), then the rest
    w1sb = wpool.tile([P, DM // P, DF], dt.bfloat16, tag="w1sb")
    w1r = w1t.rearrange("(o p) f -> p o f", p=P)
    xsb = wpool.tile([P, DM // P, ntok], dt.bfloat16, tag="xsb")
    xr = xT.rearrange("(o p) t -> p o t", p=P)
    for k in range(DM // P):
        nc.sync.dma_start(w1sb[:, k, :], w1r[:, k, :])
        nc.sync.dma_start(xsb[:, k, :], xr[:, k, :])
    csb = wpool.tile([P, nmt], dt.float32, tag="csb")
    nc.sync.dma_start(csb[:], cv.rearrange("(t p) -> p t", p=P))
    scatsb = wpool.tile([P, nmt], dt.int32, tag="scatsb")
    nc.sync.dma_start(scatsb[:], scat.rearrange("(t p) -> p t", p=P))
    g0sb = wpool.tile([P, OWN // P], dt.int32, tag="g0sb")
    nc.sync.dma_start(g0sb[:], g0.rearrange("(t p) -> p t", p=P))
    g1sb = wpool.tile([P, OWN // P], dt.int32, tag="g1sb")
    nc.sync.dma_start(g1sb[:], g1.rearrange("(t p) -> p t", p=P))
    zrow = wpool.tile([1, DM], dt.bfloat16, tag="zrow")
    nc.vector.memset(zrow[:], 0.0)
    for s in range(NSEG):
        nc.sync.dma_start(recv[s][8 * SS[s] : 8 * SS[s] + 1, :], zrow[:])
    w2sb = wpool.tile([P, DF // P, DM], dt.bfloat16, tag="w2sb")
    w2r = w2t.rearrange("(o p) d -> p o d", p=P)
    for f in range(DF // P):
        nc.sync.dma_start(w2sb[:, f, :], w2r[:, f, :])

    # ---- combine for owned m-tile s: gather 2 partial rows, add, write out
    def combine(s):
        ga = gpool.tile([P, DM], dt.bfloat16, tag="ga")
        nc.gpsimd.indirect_dma_start(
            out=ga[:],
            out_offset=None,
            in_=recv[s][:],
            in_offset=bass.IndirectOffsetOnAxis(ap=g0sb[:, s : s + 1], axis=0),
        )
        gb = gpool.tile([P, DM], dt.bfloat16, tag="gb")
        nc.gpsimd.indirect_dma_start(
            out=gb[:],
            out_offset=None,
            in_=recv[s][:],
            in_offset=bass.IndirectOffsetOnAxis(ap=g1sb[:, s : s + 1], axis=0),
        )
        ys = gpool.tile([P, DM], dt.float32, tag="ys")
        nc.vector.tensor_add(ys[:], ga[:], gb[:])
        nc.sync.dma_start(yout[s * P : (s + 1) * P, :], ys[:])

    a2a_next = 0

    def maybe_fire(tm):
        nonlocal a2a_next
        while a2a_next < NSEG and FI[a2a_next] == tm:
            s = a2a_next
            nc.gpsimd.collective_compute(
                "AllToAll",
                mybir.AluOpType.bypass,
                replica_groups=[list(range(NCORES))],
                ins=[sendbuf[RS[s] : RS[s] + 8 * SS[s], :]],
                outs=[recv[s][0 : 8 * SS[s], :]],
            )
            a2a_next += 1
            combine(s)

    # ---- expert MLP over 512-column chunks of the compact token axis ----
    for c0, csz in _chunks_of(ntok):
        hs = []
        for i in range(DF // P):  # f-tiles: H[f] = silu(w1 . x)
            ph = phpool.tile([P, csz], dt.float32, tag="ph")
            for k in range(DM // P):
                nc.tensor.matmul(
                    ph[:],
                    lhsT=w1sb[:, k, i * P : (i + 1) * P],
                    rhs=xsb[:, k, c0 : c0 + csz],
                    start=(k == 0),
                    stop=(k == DM // P - 1),
                )
            h_i = hpool.tile([P, csz], dt.bfloat16, tag="h")
            nc.scalar.activation(h_i[:], ph[:], mybir.ActivationFunctionType.Silu)
            hs.append(h_i)
        for tml in range(csz // P):
            tm = c0 // P + tml
            py0 = pypool.tile([P, 512], dt.float32, tag="py")
            py1 = pypool.tile([P, 512], dt.float32, tag="py")
            for f in range(DF // P):
                lhs = hs[f][:, tml * P : (tml + 1) * P]
                nc.tensor.matmul(
                    py0[:], lhsT=lhs, rhs=w2sb[:, f, 0:512],
                    start=(f == 0), stop=(f == DF // P - 1),
                )
                nc.tensor.matmul(
                    py1[:], lhsT=lhs, rhs=w2sb[:, f, 512:1024],
                    start=(f == 0), stop=(f == DF // P - 1),
                )
            y_sb = ypool.tile([P, DM], dt.bfloat16, tag="y")
            nc.vector.tensor_scalar_mul(y_sb[:, 0:512], py0[:], csb[:, tm : tm + 1])
            nc.vector.tensor_scalar_mul(
                y_sb[:, 512:1024], py1[:], csb[:, tm : tm + 1]
            )
            nc.gpsimd.indirect_dma_start(
                out=sendbuf[0 : BND[tm], :],
                out_offset=bass.IndirectOffsetOnAxis(
                    ap=scatsb[:, tm : tm + 1], axis=0
                ),
                in_=y_sb[:],
            )
            maybe_fire(tm)
    assert a2a_next == NSEG, (a2a_next, FI, nmt)


def _build_program(plan: dict):
    key = ("v2", plan["ntok"], tuple(plan["SS"]), tuple(plan["FI"]), tuple(plan["BND"]))
    if key in _PROGRAM_CACHE:
        return _PROGRAM_CACHE[key]
    from contextlib import ExitStack

    nc = bacc.Bacc(
        "TRN2",
        target_bir_lowering=False,
        debug=False,
        enable_asserts=True,
        num_devices=NCORES,
    )
    with tile.TileContext(nc) as tc:
        with ExitStack() as ctx:
            _emit(nc, tc, ctx, plan)
    nc.compile()
    _PROGRAM_CACHE[key] = nc
    return nc


def prepare_in_maps(x, topk_e, topk_w, w1, w2):
    """Host-side routing/dispatch. Returns (in_maps, plan)."""
    bf16 = ml_dtypes.bfloat16
    x = np.asarray(x)
    topk_e = np.asarray(topk_e)
    topk_w = np.asarray(topk_w)
    w1 = np.asarray(w1)
    w2 = np.asarray(w2)

    # per-token combine weight for each expert
    c = np.zeros((S, E), dtype=np.float32)
    np.add.at(c, (np.arange(S)[:, None], topk_e), topk_w.astype(np.float32))

    toks = [np.nonzero((topk_e == e).any(axis=1))[0] for e in range(E)]
    # group counts per (expert, owner, seg)
    cnt = np.zeros((E, NCORES, NSEG), dtype=np.int64)
    for e in range(E):
        d = toks[e] // OWN
        sg = (toks[e] % OWN) // OWNSEG
        np.add.at(cnt[e], (d, sg), 1)
    SS = [int(v) for v in cnt.max(axis=(0, 1))]
    seg_tot = cnt.sum(axis=1)  # [E, NSEG] tokens per (expert, seg)
    cum = np.concatenate(
        [np.zeros((E, 1), np.int64), np.cumsum(seg_tot, axis=1)], axis=1
    )  # [E, NSEG+1]
    n_e = cum[:, -1]
    ntok = int(-(-n_e.max() // P) * P)
    nmt = ntok // P
    FI = [min(int(-(-cum[:, s + 1].max() // P)) - 1, nmt - 1) for s in range(NSEG)]
    # reverse-order segment layout: trash rows [0,P), then seg NSEG-1 .. seg 0
    RS = [P + 8 * int(sum(SS[s + 1 :])) for s in range(NSEG)]
    # per-m-tile scatter AP bound: prefix through the lowest seg present
    BND = []
    for tm in range(nmt):
        slo = NSEG - 1
        pos = tm * P
        for e in range(E):
            if pos < n_e[e]:
                sfound = int(np.searchsorted(cum[e], pos, side="right")) - 1
                slo = min(slo, sfound)
        BND.append(RS[slo] + 8 * SS[slo])
    plan = {"ntok": ntok, "SS": SS, "RS": RS, "FI": FI, "BND": BND}

    # per-core compact dispatch + recv-row map
    rr = np.full((E, S), -1, dtype=np.int64)  # (expert, token) -> row in recv[seg]
    in_maps = []
    for e in range(E):
        te = toks[e]
        d = te // OWN
        sg = (te % OWN) // OWNSEG
        order = np.lexsort((te, d, sg))  # by (seg, owner, token)
        te_o, d_o, s_o = te[order], d[order], sg[order]
        gid = s_o * NCORES + d_o  # non-decreasing in compact order
        rank = np.arange(len(te_o)) - np.searchsorted(gid, gid, side="left")
        ss_o = np.array(SS)[s_o]
        rs_o = np.array(RS)[s_o]
        send_row = rs_o + d_o * ss_o + rank
        rr[e, te_o] = e * ss_o + rank

        ne = len(te_o)
        xT_e = np.zeros((DM, ntok), dtype=bf16)
        xT_e[:, :ne] = x[te_o].T.astype(bf16)
        cv_e = np.zeros(ntok, dtype=np.float32)
        cv_e[:ne] = c[te_o, e]
        scat_e = (np.arange(ntok) % P).astype(np.int32)  # pads -> trash rows
        scat_e[:ne] = send_row.astype(np.int32)
        in_maps.append(
            {
                "xT": xT_e,
                "cv": cv_e,
                "scat": scat_e,
                "w1t": np.ascontiguousarray(w1[e].T).astype(bf16),
                "w2t": np.ascontiguousarray(w2[e].T).astype(bf16),
            }
        )

    # owner-side gather indices
    all_t = np.arange(S)
    e0 = topk_e[:, 0]
    e1 = topk_e[:, 1]
    sg_all = (all_t % OWN) // OWNSEG
    zero_row = 8 * np.array(SS)[sg_all]
    g0_all = rr[e0, all_t]
    g1_all = np.where(e0 == e1, zero_row, rr[e1, all_t])
    assert (g0_all >= 0).all() and (g1_all >= 0).all()
    for dcore in range(NCORES):
        sl = slice(dcore * OWN, (dcore + 1) * OWN)
        in_maps[dcore]["g0"] = g0_all[sl].astype(np.int32)
        in_maps[dcore]["g1"] = g1_all[sl].astype(np.int32)

    return in_maps, plan


def kernel(x, topk_e, topk_w, w1, w2):
    in_maps, plan = prepare_in_maps(x, topk_e, topk_w, w1, w2)
    nc = _build_program(plan)
    res = run_bass_kernel_spmd(nc, in_maps, list(range(NCORES)))
    out = np.concatenate([res.results[d]["yout"] for d in range(NCORES)], axis=0)
    return out.astype(np.float32)


# revision 3
# speedup vs baseline: 104562.2492x; 1.1043x over previous
"""Expert-parallel MoE MLP (top-2 of 8 experts) on 8 TRN2 NeuronCores.

Strategy (expert-parallel, per sharding hint):
  - core e holds expert e's weights (w1[e], w2[e], host-pre-transposed, bf16)
  - host dispatches tokens by expert id into a COMPACT layout: core e's
    xT holds exactly its routed tokens (ordered by owned-segment, then
    owner, then token id), padded only at the tail to a multiple of 128.
    This minimizes matmul columns (vs block-padded dispatch).
  - core e computes y = [silu(x_e @ w1[e]^T) * c_e] @ w2[e]^T over
    512-column chunks (bf16 matmuls, fp32 accumulate).
  - mm2 output m-tiles are indirect-scattered into `sendbuf`, which holds
    the AllToAll wire layout: NSEG slot-segments, stored in REVERSE
    segment order (plus a leading trash region for pad rows) so that each
    scatter's AP byte-extent is a prefix that never overlaps the A2A read
    slices of earlier-fired segments (avoids false WAR serialization).
  - NSEG chunked AllToAlls fire as soon as the last m-tile holding each
    segment's tokens has been scattered; segment s holds the partial rows
    of owned-token m-tile s on every owner, so the owner-side combine
    (2 indirect row-gathers + add) runs right after each A2A lands,
    overlapped with remaining compute and later A2As.
  - 16 dummy matmuls at t=0 warm the PE HAM clock gate during the loads.
"""

import sys

sys.path.insert(0, "/opt/trn_rl_repo")

import numpy as np
import ml_dtypes

import concourse.bass as bass
import concourse.tile as tile
from concourse import bacc, mybir
from concourse.bass_utils import run_bass_kernel_spmd

S, DM, DF, E, TOPK = 4096, 1024, 2048, 8, 2
NCORES = 8
P = 128
OWN = S // NCORES  # tokens per owner core
OWNSEG = 128  # owned tokens per A2A segment (= one combine m-tile)
NSEG = OWN // OWNSEG

_PROGRAM_CACHE: dict = {}


def _chunks_of(ntok: int) -> list[tuple[int, int]]:
    """Split ntok into (start, size) chunks, each a multiple of 128, <= 512."""
    out, pos = [], 0
    while pos < ntok:
        sz = min(512, ntok - pos)
        out.append((pos, sz))
        pos += sz
    return out


def _emit(nc, tc, ctx, plan: dict):
    dt = mybir.dt
    ntok = plan["ntok"]
    SS = plan["SS"]  # per-seg slot-block size
    RS = plan["RS"]  # per-seg row start in sendbuf (reverse layout)
    FI = plan["FI"]  # fire A2A-s after scatter of m-tile FI[s]
    BND = plan["BND"]  # per-m-tile scatter AP row bound (prefix extent)
    nmt = ntok // P
    blk = sum(SS)
    sendrows = P + 8 * blk  # leading P trash rows + segments

    xT = nc.dram_tensor("xT", [DM, ntok], dt.bfloat16, kind="ExternalInput").ap()
    w1t = nc.dram_tensor("w1t", [DM, DF], dt.bfloat16, kind="ExternalInput").ap()
    w2t = nc.dram_tensor("w2t", [DF, DM], dt.bfloat16, kind="ExternalInput").ap()
    cv = nc.dram_tensor("cv", [ntok], dt.float32, kind="ExternalInput").ap()
    scat = nc.dram_tensor("scat", [ntok], dt.int32, kind="ExternalInput").ap()
    g0 = nc.dram_tensor("g0", [OWN], dt.int32, kind="ExternalInput").ap()
    g1 = nc.dram_tensor("g1", [OWN], dt.int32, kind="ExternalInput").ap()
    yout = nc.dram_tensor("yout", [OWN, DM], dt.float32, kind="ExternalOutput").ap()
    sendbuf = nc.dram_tensor("sendbuf", [sendrows, DM], dt.bfloat16).ap()
    recv = [
        nc.dram_tensor(f"recv{s}", [8 * SS[s] + 1, DM], dt.bfloat16).ap()
        for s in range(NSEG)
    ]

    wpool = ctx.enter_context(tc.tile_pool(name="w", bufs=1))
    hpool = ctx.enter_context(tc.tile_pool(name="h", bufs=34))
    ypool = ctx.enter_context(tc.tile_pool(name="y", bufs=6))
    gpool = ctx.enter_context(tc.tile_pool(name="g", bufs=2))
    phpool = ctx.enter_context(tc.tile_pool(name="ph", bufs=2, space="PSUM"))
    pypool = ctx.enter_context(tc.tile_pool(name="py", bufs=4, space="PSUM"))
    pwpool = ctx.enter_context(tc.tile_pool(name="pw", bufs=1, space="PSUM"))

    # ---- PE warmup: release the HAM clock gate while DMAs load ----
    warm = wpool.tile([P, 512], dt.bfloat16, tag="warm")
    nc.vector.memset(warm[:], 0.0)
    pw = pwpool.tile([P, 512], dt.float32, tag="pw")
    for _ in range(16):
        nc.tensor.matmul(pw[:], lhsT=warm[:, 0:P], rhs=warm[:], start=True, stop=True)

    # ---- loads: w1/x interleaved (first matmuls need them), then the rest
    w1sb = wpool.tile([P, DM // P, DF], dt.bfloat16, tag="w1sb")
    w1r = w1t.rearrange("(o p) f -> p o f", p=P)
    xsb = wpool.tile([P, DM // P, ntok], dt.bfloat16, tag="xsb")
    xr = xT.rearrange("(o p) t -> p o t", p=P)
    for k in range(DM // P):
        nc.sync.dma_start(w1sb[:, k, :], w1r[:, k, :])
        nc.sync.dma_start(xsb[:, k, :], xr[:, k, :])
    csb = wpool.tile([P, nmt], dt.float32, tag="csb")
    nc.sync.dma_start(csb[:], cv.rearrange("(t p) -> p t", p=P))
    scatsb = wpool.tile([P, nmt], dt.int32, tag="scatsb")
    nc.sync.dma_start(scatsb[:], scat.rearrange("(t p) -> p t", p=P))
    g0sb = wpool.tile([P, OWN // P], dt.int32, tag="g0sb")
    nc.sync.dma_start(g0sb[:], g0.rearrange("(t p) -> p t", p=P))
    g1sb = wpool.tile([P, OWN // P], dt.int32, tag="g1sb")
    nc.sync.dma_start(g1sb[:], g1.rearrange("(t p) -> p t", p=P))
    zrow = wpool.tile([1, DM], dt.bfloat16, tag="zrow")
    nc.vector.memset(zrow[:], 0.0)
    for s in range(NSEG):
        nc.sync.dma_start(recv[s][8 * SS[s] : 8 * SS[s] + 1, :], zrow[:])
    w2sb = wpool.tile([P, DF // P, DM], dt.bfloat16, tag="w2sb")
    w2r = w2t.rearrange("(o p) d -> p o d", p=P)
    for f in range(DF // P):
        nc.sync.dma_start(w2sb[:, f, :], w2r[:, f, :])

    # ---- combine for owned m-tile s: gather 2 partial rows, add, write out
    def combine(s):
        ga = gpool.tile([P, DM], dt.bfloat16, tag="ga")
        nc.gpsimd.indirect_dma_start(
            out=ga[:],
            out_offset=None,
            in_=recv[s][:],
            in_offset=bass.IndirectOffsetOnAxis(ap=g0sb[:, s : s + 1], axis=0),
        )
        gb = gpool.tile([P, DM], dt.bfloat16, tag="gb")
        nc.gpsimd.indirect_dma_start(
            out=gb[:],
            out_offset=None,
            in_=recv[s][:],
            in_offset=bass.IndirectOffsetOnAxis(ap=g1sb[:, s : s + 1], axis=0),
        )
        ys = gpool.tile([P, DM], dt.float32, tag="ys")
        nc.vector.tensor_add(ys[:], ga[:], gb[:])
        nc.sync.dma_start(yout[s * P : (s + 1) * P, :], ys[:])

    a2a_next = 0

    def maybe_fire(tm):
        nonlocal a2a_next
        while a2a_next < NSEG and FI[a2a_next] == tm:
            s = a2a_next
            nc.gpsimd.collective_compute(
                "AllToAll",
                mybir.AluOpType.bypass,
                replica_groups=[list(range(NCORES))],
                ins=[sendbuf[RS[s] : RS[s] + 8 * SS[s], :]],
                outs=[recv[s][0 : 8 * SS[s], :]],
            )
            a2a_next += 1
            combine(s)

    # ---- expert MLP over 512-column chunks of the compact token axis ----
    for c0, csz in _chunks_of(ntok):
        hs = []
        for i in range(DF // P):  # f-tiles: H[f] = silu(w1 . x)
            ph = phpool.tile([P, csz], dt.float32, tag="ph")
            for k in range(DM // P):
                nc.tensor.matmul(
                    ph[:],
                    lhsT=w1sb[:, k, i * P : (i + 1) * P],
                    rhs=xsb[:, k, c0 : c0 + csz],
                    start=(k == 0),
                    stop=(k == DM // P - 1),
                )
            h_i = hpool.tile([P, csz], dt.bfloat16, tag="h")
            nc.scalar.activation(h_i[:], ph[:], mybir.ActivationFunctionType.Silu)
            hs.append(h_i)
        for tml in range(csz // P):
            tm = c0 // P + tml
            py0 = pypool.tile([P, 512], dt.float32, tag="py")
            py1 = pypool.tile([P, 512], dt.float32, tag="py")
            for f in range(DF // P):
                lhs = hs[f][:, tml * P : (tml + 1) * P]
                nc.tensor.matmul(
                    py0[:], lhsT=lhs, rhs=w2sb[:, f, 0:512],
                    start=(f == 0), stop=(f == DF // P - 1),
                )
                nc.tensor.matmul(
                    py1[:], lhsT=lhs, rhs=w2sb[:, f, 512:1024],
                    start=(f == 0), stop=(f == DF // P - 1),
                )
            y_sb = ypool.tile([P, DM], dt.bfloat16, tag="y")
            nc.vector.tensor_scalar_mul(y_sb[:, 0:512], py0[:], csb[:, tm : tm + 1])
            nc.vector.tensor_scalar_mul(
                y_sb[:, 512:1024], py1[:], csb[:, tm : tm + 1]
            )
            nc.gpsimd.indirect_dma_start(
                out=sendbuf[0 : BND[tm], :],
                out_offset=bass.IndirectOffsetOnAxis(
                    ap=scatsb[:, tm : tm + 1], axis=0
                ),
                in_=y_sb[:],
                in_offset=None,
            )
            maybe_fire(tm)
    assert a2a_next == NSEG, (a2a_next, FI, nmt)


def _build_program(plan: dict):
    key = ("v2", plan["ntok"], tuple(plan["SS"]), tuple(plan["FI"]), tuple(plan["BND"]))
    if key in _PROGRAM_CACHE:
        return _PROGRAM_CACHE[key]
    from contextlib import ExitStack

    nc = bacc.Bacc(
        "TRN2",
        target_bir_lowering=False,
        debug=False,
        enable_asserts=True,
        num_devices=NCORES,
    )
    with tile.TileContext(nc) as tc:
        with ExitStack() as ctx:
            _emit(nc, tc, ctx, plan)
    nc.compile()
    _PROGRAM_CACHE[key] = nc
    return nc


def prepare_in_maps(x, topk_e, topk_w, w1, w2):
    """Host-side routing/dispatch. Returns (in_maps, plan)."""
    bf16 = ml_dtypes.bfloat16
    x = np.asarray(x)
    topk_e = np.asarray(topk_e)
    topk_w = np.asarray(topk_w)
    w1 = np.asarray(w1)
    w2 = np.asarray(w2)

    # per-token combine weight for each expert
    c = np.zeros((S, E), dtype=np.float32)
    np.add.at(c, (np.arange(S)[:, None], topk_e), topk_w.astype(np.float32))

    toks = [np.nonzero((topk_e == e).any(axis=1))[0] for e in range(E)]
    # group counts per (expert, owner, seg)
    cnt = np.zeros((E, NCORES, NSEG), dtype=np.int64)
    for e in range(E):
        d = toks[e] // OWN
        sg = (toks[e] % OWN) // OWNSEG
        np.add.at(cnt[e], (d, sg), 1)
    SS = [int(v) for v in cnt.max(axis=(0, 1))]
    seg_tot = cnt.sum(axis=1)  # [E, NSEG] tokens per (expert, seg)
    cum = np.concatenate(
        [np.zeros((E, 1), np.int64), np.cumsum(seg_tot, axis=1)], axis=1
    )  # [E, NSEG+1]
    n_e = cum[:, -1]
    ntok = int(-(-n_e.max() // P) * P)
    nmt = ntok // P
    FI = [min(int(-(-cum[:, s + 1].max() // P)) - 1, nmt - 1) for s in range(NSEG)]
    # reverse-order segment layout: trash rows [0,P), then seg NSEG-1 .. seg 0
    RS = [P + 8 * int(sum(SS[s + 1 :])) for s in range(NSEG)]
    # per-m-tile scatter AP bound: prefix through the lowest seg present
    BND = []
    for tm in range(nmt):
        slo = NSEG - 1
        pos = tm * P
        for e in range(E):
            if pos < n_e[e]:
                sfound = int(np.searchsorted(cum[e], pos, side="right")) - 1
                slo = min(slo, sfound)
        BND.append(RS[slo] + 8 * SS[slo])
    plan = {"ntok": ntok, "SS": SS, "RS": RS, "FI": FI, "BND": BND}

    # per-core compact dispatch + recv-row map
    rr = np.full((E, S), -1, dtype=np.int64)  # (expert, token) -> row in recv[seg]
    in_maps = []
    for e in range(E):
        te = toks[e]
        d = te // OWN
        sg = (te % OWN) // OWNSEG
        order = np.lexsort((te, d, sg))  # by (seg, owner, token)
        te_o, d_o, s_o = te[order], d[order], sg[order]
        gid = s_o * NCORES + d_o  # non-decreasing in compact order
        rank = np.arange(len(te_o)) - np.searchsorted(gid, gid, side="left")
        ss_o = np.array(SS)[s_o]
        rs_o = np.array(RS)[s_o]
        send_row = rs_o + d_o * ss_o + rank
        rr[e, te_o] = e * ss_o + rank

        ne = len(te_o)
        xT_e = np.zeros((DM, ntok), dtype=bf16)
        xT_e[:, :ne] = x[te_o].T.astype(bf16)
        cv_e = np.zeros(ntok, dtype=np.float32)
        cv_e[:ne] = c[te_o, e]
        scat_e = (np.arange(ntok) % P).astype(np.int32)  # pads -> trash rows
        scat_e[:ne] = send_row.astype(np.int32)
        in_maps.append(
            {
                "xT": xT_e,
                "cv": cv_e,
                "scat": scat_e,
                "w1t": np.ascontiguousarray(w1[e].T).astype(bf16),
                "w2t": np.ascontiguousarray(w2[e].T).astype(bf16),
            }
        )

    # owner-side gather indices
    all_t = np.arange(S)
    e0 = topk_e[:, 0]
    e1 = topk_e[:, 1]
    sg_all = (all_t % OWN) // OWNSEG
    zero_row = 8 * np.array(SS)[sg_all]
    g0_all = rr[e0, all_t]
    g1_all = np.where(e0 == e1, zero_row, rr[e1, all_t])
    assert (g0_all >= 0).all() and (g1_all >= 0).all()
    for dcore in range(NCORES):
        sl = slice(dcore * OWN, (dcore + 1) * OWN)
        in_maps[dcore]["g0"] = g0_all[sl].astype(np.int32)
        in_maps[dcore]["g1"] = g1_all[sl].astype(np.int32)

    return in_maps, plan


def kernel(x, topk_e, topk_w, w1, w2):
    in_maps, plan = prepare_in_maps(x, topk_e, topk_w, w1, w2)
    nc = _build_program(plan)
    res = run_bass_kernel_spmd(nc, in_maps, list(range(NCORES)))
    out = np.concatenate([res.results[d]["yout"] for d in range(NCORES)], axis=0)
    return out.astype(np.float32)


# revision 5
# speedup vs baseline: 112536.8481x; 1.0763x over previous
"""Expert-parallel MoE MLP (top-2 of 8 experts) on 8 TRN2 NeuronCores.

Strategy (expert-parallel, per sharding hint):
  - core e holds expert e's weights (w1[e], w2[e], host-pre-transposed, bf16)
  - host dispatches tokens by expert id into a COMPACT layout: core e's
    xT holds exactly its routed tokens (ordered by owned-segment, then
    owner, then token id), padded only at the tail to a multiple of 128.
    This minimizes matmul columns (vs block-padded dispatch).
  - core e computes y = [silu(x_e @ w1[e]^T) * c_e] @ w2[e]^T over
    512-column chunks (bf16 matmuls, fp32 accumulate).
  - mm2 output m-tiles are indirect-scattered into `sendbuf`, which holds
    the AllToAll wire layout: NSEG slot-segments, stored in REVERSE
    segment order (plus a leading trash region for pad rows) so that each
    scatter's AP byte-extent is a prefix that never overlaps the A2A read
    slices of earlier-fired segments (avoids false WAR serialization).
  - NSEG chunked AllToAlls fire as soon as the last m-tile holding each
    segment's tokens has been scattered; segment s holds the partial rows
    of owned-token m-tile s on every owner, so the owner-side combine
    (2 indirect row-gathers + add) runs right after each A2A lands,
    overlapped with remaining compute and later A2As.
  - 16 dummy matmuls at t=0 warm the PE HAM clock gate during the loads.
"""

import sys

sys.path.insert(0, "/opt/trn_rl_repo")

import numpy as np
import ml_dtypes

import concourse.bass as bass
import concourse.tile as tile
from concourse import bacc, mybir
from concourse.bass_utils import run_bass_kernel_spmd

S, DM, DF, E, TOPK = 4096, 1024, 2048, 8, 2
NCORES = 8
P = 128
OWN = S // NCORES  # tokens per owner core
OWNSEG = 128  # owned tokens per A2A segment (= one combine m-tile)
NSEG = OWN // OWNSEG

_PROGRAM_CACHE: dict = {}


def _chunks_of(ntok: int) -> list[tuple[int, int]]:
    """Split ntok into (start, size) chunks, each a multiple of 128, <= 512."""
    out, pos = [], 0
    while pos < ntok:
        sz = min(512, ntok - pos)
        out.append((pos, sz))
        pos += sz
    return out


def _emit(nc, tc, ctx, plan: dict):
    dt = mybir.dt
    ntok = plan["ntok"]
    SS = plan["SS"]  # per-seg slot-block size
    RS = plan["RS"]  # per-seg row start in sendbuf (reverse layout)
    FI = plan["FI"]  # fire A2A-s after scatter of m-tile FI[s]
    BND = plan["BND"]  # per-m-tile scatter AP row bound (prefix extent)
    nmt = ntok // P
    blk = sum(SS)
    sendrows = P + 8 * blk  # leading P trash rows + segments

    xT = nc.dram_tensor("xT", [DM, ntok], dt.bfloat16, kind="ExternalInput").ap()
    w1t = nc.dram_tensor("w1t", [DM, DF], dt.bfloat16, kind="ExternalInput").ap()
    w2t = nc.dram_tensor("w2t", [DF, DM], dt.bfloat16, kind="ExternalInput").ap()
    cv = nc.dram_tensor("cv", [ntok], dt.float32, kind="ExternalInput").ap()
    scat = nc.dram_tensor("scat", [ntok], dt.int32, kind="ExternalInput").ap()
    g0 = nc.dram_tensor("g0", [OWN], dt.int32, kind="ExternalInput").ap()
    g1 = nc.dram_tensor("g1", [OWN], dt.int32, kind="ExternalInput").ap()
    yout = nc.dram_tensor("yout", [OWN, DM], dt.float32, kind="ExternalOutput").ap()
    sendbuf = nc.dram_tensor("sendbuf", [sendrows, DM], dt.bfloat16).ap()
    recv = [
        nc.dram_tensor(f"recv{s}", [8 * SS[s] + 1, DM], dt.bfloat16).ap()
        for s in range(NSEG)
    ]

    dsend = nc.dram_tensor("dsend", [NCORES, 64], dt.bfloat16).ap()
    drecv = nc.dram_tensor("drecv", [NCORES, 64], dt.bfloat16).ap()

    wpool = ctx.enter_context(tc.tile_pool(name="w", bufs=1))
    hpool = ctx.enter_context(tc.tile_pool(name="h", bufs=34))
    ypool = ctx.enter_context(tc.tile_pool(name="y", bufs=10))
    gpool = ctx.enter_context(tc.tile_pool(name="g", bufs=2))
    phpool = ctx.enter_context(tc.tile_pool(name="ph", bufs=2, space="PSUM"))
    pypool = ctx.enter_context(tc.tile_pool(name="py", bufs=4, space="PSUM"))
    pwpool = ctx.enter_context(tc.tile_pool(name="pw", bufs=1, space="PSUM"))

    # ---- dummy collective: absorb cross-core start skew + ncfw first-call
    # cost while loads/compute proceed (its data is never consumed)
    nc.gpsimd.collective_compute(
        "AllToAll",
        mybir.AluOpType.bypass,
        replica_groups=[list(range(NCORES))],
        ins=[dsend[:, :]],
        outs=[drecv[:, :]],
    )

    # ---- PE warmup: release the HAM clock gate while DMAs load ----
    warm = wpool.tile([P, 512], dt.bfloat16, tag="warm")
    nc.vector.memset(warm[:], 0.0)
    pw = pwpool.tile([P, 512], dt.float32, tag="pw")
    for _ in range(16):
        nc.tensor.matmul(pw[:], lhsT=warm[:, 0:P], rhs=warm[:], start=True, stop=True)

    # ---- loads: few big DMAs, ordered by first use. mm1's i-loop needs all
    # k-tiles of w1 for one 128-wide f-slice, so load w1 f-major.
    w1sb = wpool.tile([P, DM // P, DF], dt.bfloat16, tag="w1sb")
    w1r = w1t.rearrange("(o p) f -> p o f", p=P)
    xsb = wpool.tile([P, DM // P, ntok], dt.bfloat16, tag="xsb")
    xr = xT.rearrange("(o p) t -> p o t", p=P)
    nc.sync.dma_start(w1sb[:, :, 0:512], w1r[:, :, 0:512])
    nc.sync.dma_start(xsb[:, :, 0 : min(512, ntok)], xr[:, :, 0 : min(512, ntok)])
    for fs in range(512, DF, 512):
        nc.sync.dma_start(w1sb[:, :, fs : fs + 512], w1r[:, :, fs : fs + 512])
    if ntok > 512:
        nc.sync.dma_start(xsb[:, :, 512:ntok], xr[:, :, 512:ntok])
    w2sb = wpool.tile([P, DF // P, DM], dt.bfloat16, tag="w2sb")
    w2r = w2t.rearrange("(o p) d -> p o d", p=P)
    nc.sync.dma_start(w2sb[:, 0:8, :], w2r[:, 0:8, :])
    nc.sync.dma_start(w2sb[:, 8:16, :], w2r[:, 8:16, :])
    csb = wpool.tile([P, nmt], dt.float32, tag="csb")
    nc.sync.dma_start(csb[:], cv.rearrange("(t p) -> p t", p=P))
    scatsb = wpool.tile([P, nmt], dt.int32, tag="scatsb")
    nc.sync.dma_start(scatsb[:], scat.rearrange("(t p) -> p t", p=P))
    g0sb = wpool.tile([P, OWN // P], dt.int32, tag="g0sb")
    nc.sync.dma_start(g0sb[:], g0.rearrange("(t p) -> p t", p=P))
    g1sb = wpool.tile([P, OWN // P], dt.int32, tag="g1sb")
    nc.sync.dma_start(g1sb[:], g1.rearrange("(t p) -> p t", p=P))
    zrow = wpool.tile([1, DM], dt.bfloat16, tag="zrow")
    nc.vector.memset(zrow[:], 0.0)
    for s in range(NSEG):
        nc.sync.dma_start(recv[s][8 * SS[s] : 8 * SS[s] + 1, :], zrow[:])

    # ---- combine for owned m-tile s: gather 2 partial rows, add, write out
    def combine(s):
        ga = gpool.tile([P, DM], dt.bfloat16, tag="ga")
        nc.gpsimd.indirect_dma_start(
            out=ga[:],
            out_offset=None,
            in_=recv[s][:],
            in_offset=bass.IndirectOffsetOnAxis(ap=g0sb[:, s : s + 1], axis=0),
        )
        gb = gpool.tile([P, DM], dt.bfloat16, tag="gb")
        nc.gpsimd.indirect_dma_start(
            out=gb[:],
            out_offset=None,
            in_=recv[s][:],
            in_offset=bass.IndirectOffsetOnAxis(ap=g1sb[:, s : s + 1], axis=0),
        )
        ys = gpool.tile([P, DM], dt.float32, tag="ys")
        nc.vector.tensor_add(ys[:], ga[:], gb[:])
        nc.sync.dma_start(yout[s * P : (s + 1) * P, :], ys[:])

    a2a_next = 0

    def maybe_fire(tm):
        nonlocal a2a_next
        while a2a_next < NSEG and FI[a2a_next] == tm:
            s = a2a_next
            nc.gpsimd.collective_compute(
                "AllToAll",
                mybir.AluOpType.bypass,
                replica_groups=[list(range(NCORES))],
                ins=[sendbuf[RS[s] : RS[s] + 8 * SS[s], :]],
                outs=[recv[s][0 : 8 * SS[s], :]],
            )
            a2a_next += 1
            combine(s)

    # ---- expert MLP over 512-column chunks of the compact token axis ----
    for c0, csz in _chunks_of(ntok):
        hs = []
        for i in range(DF // P):  # f-tiles: H[f] = silu(w1 . x)
            ph = phpool.tile([P, csz], dt.float32, tag="ph")
            for k in range(DM // P):
                nc.tensor.matmul(
                    ph[:],
                    lhsT=w1sb[:, k, i * P : (i + 1) * P],
                    rhs=xsb[:, k, c0 : c0 + csz],
                    start=(k == 0),
                    stop=(k == DM // P - 1),
                )
            h_i = hpool.tile([P, csz], dt.bfloat16, tag="h")
            nc.scalar.activation(h_i[:], ph[:], mybir.ActivationFunctionType.Silu)
            hs.append(h_i)
        for tml in range(csz // P):
            tm = c0 // P + tml
            py0 = pypool.tile([P, 512], dt.float32, tag="py")
            py1 = pypool.tile([P, 512], dt.float32, tag="py")
            for f in range(DF // P):
                lhs = hs[f][:, tml * P : (tml + 1) * P]
                nc.tensor.matmul(
                    py0[:], lhsT=lhs, rhs=w2sb[:, f, 0:512],
                    start=(f == 0), stop=(f == DF // P - 1),
                )
                nc.tensor.matmul(
                    py1[:], lhsT=lhs, rhs=w2sb[:, f, 512:1024],
                    start=(f == 0), stop=(f == DF // P - 1),
                )
            y_sb = ypool.tile([P, DM], dt.bfloat16, tag="y")
            nc.vector.tensor_scalar_mul(y_sb[:, 0:512], py0[:], csb[:, tm : tm + 1])
            nc.vector.tensor_scalar_mul(
                y_sb[:, 512:1024], py1[:], csb[:, tm : tm + 1]
            )
            nc.gpsimd.indirect_dma_start(
                out=sendbuf[0 : BND[tm], :],
                out_offset=bass.IndirectOffsetOnAxis(
                    ap=scatsb[:, tm : tm + 1], axis=0
                ),
                in_=y_sb[:],
                in_offset=None,
            )
            maybe_fire(tm)
    assert a2a_next == NSEG, (a2a_next, FI, nmt)


def _build_program(plan: dict):
    key = ("v2", plan["ntok"], tuple(plan["SS"]), tuple(plan["FI"]), tuple(plan["BND"]))
    if key in _PROGRAM_CACHE:
        return _PROGRAM_CACHE[key]
    from contextlib import ExitStack

    nc = bacc.Bacc(
        "TRN2",
        target_bir_lowering=False,
        debug=False,
        enable_asserts=True,
        num_devices=NCORES,
    )
    with tile.TileContext(nc) as tc:
        with ExitStack() as ctx:
            _emit(nc, tc, ctx, plan)
    nc.compile()
    _PROGRAM_CACHE[key] = nc
    return nc


def prepare_in_maps(x, topk_e, topk_w, w1, w2):
    """Host-side routing/dispatch. Returns (in_maps, plan)."""
    bf16 = ml_dtypes.bfloat16
    x = np.asarray(x)
    topk_e = np.asarray(topk_e)
    topk_w = np.asarray(topk_w)
    w1 = np.asarray(w1)
    w2 = np.asarray(w2)

    # per-token combine weight for each expert
    c = np.zeros((S, E), dtype=np.float32)
    np.add.at(c, (np.arange(S)[:, None], topk_e), topk_w.astype(np.float32))

    toks = [np.nonzero((topk_e == e).any(axis=1))[0] for e in range(E)]
    # group counts per (expert, owner, seg)
    cnt = np.zeros((E, NCORES, NSEG), dtype=np.int64)
    for e in range(E):
        d = toks[e] // OWN
        sg = (toks[e] % OWN) // OWNSEG
        np.add.at(cnt[e], (d, sg), 1)
    SS = [int(v) for v in cnt.max(axis=(0, 1))]
    seg_tot = cnt.sum(axis=1)  # [E, NSEG] tokens per (expert, seg)
    cum = np.concatenate(
        [np.zeros((E, 1), np.int64), np.cumsum(seg_tot, axis=1)], axis=1
    )  # [E, NSEG+1]
    n_e = cum[:, -1]
    ntok = int(-(-n_e.max() // P) * P)
    nmt = ntok // P
    FI = [min(int(-(-cum[:, s + 1].max() // P)) - 1, nmt - 1) for s in range(NSEG)]
    # reverse-order segment layout: trash rows [0,P), then seg NSEG-1 .. seg 0
    RS = [P + 8 * int(sum(SS[s + 1 :])) for s in range(NSEG)]
    # per-m-tile scatter AP bound: prefix through the lowest seg present
    BND = []
    for tm in range(nmt):
        slo = NSEG - 1
        pos = tm * P
        for e in range(E):
            if pos < n_e[e]:
                sfound = int(np.searchsorted(cum[e], pos, side="right")) - 1
                slo = min(slo, sfound)
        BND.append(RS[slo] + 8 * SS[slo])
    plan = {"ntok": ntok, "SS": SS, "RS": RS, "FI": FI, "BND": BND}

    # per-core compact dispatch + recv-row map
    rr = np.full((E, S), -1, dtype=np.int64)  # (expert, token) -> row in recv[seg]
    in_maps = []
    for e in range(E):
        te = toks[e]
        d = te // OWN
        sg = (te % OWN) // OWNSEG
        order = np.lexsort((te, d, sg))  # by (seg, owner, token)
        te_o, d_o, s_o = te[order], d[order], sg[order]
        gid = s_o * NCORES + d_o  # non-decreasing in compact order
        rank = np.arange(len(te_o)) - np.searchsorted(gid, gid, side="left")
        ss_o = np.array(SS)[s_o]
        rs_o = np.array(RS)[s_o]
        send_row = rs_o + d_o * ss_o + rank
        rr[e, te_o] = e * ss_o + rank

        ne = len(te_o)
        xT_e = np.zeros((DM, ntok), dtype=bf16)
        xT_e[:, :ne] = x[te_o].T.astype(bf16)
        cv_e = np.zeros(ntok, dtype=np.float32)
        cv_e[:ne] = c[te_o, e]
        scat_e = (np.arange(ntok) % P).astype(np.int32)  # pads -> trash rows
        scat_e[:ne] = send_row.astype(np.int32)
        in_maps.append(
            {
                "xT": xT_e,
                "cv": cv_e,
                "scat": scat_e,
                "w1t": np.ascontiguousarray(w1[e].T).astype(bf16),
                "w2t": np.ascontiguousarray(w2[e].T).astype(bf16),
            }
        )

    # owner-side gather indices
    all_t = np.arange(S)
    e0 = topk_e[:, 0]
    e1 = topk_e[:, 1]
    sg_all = (all_t % OWN) // OWNSEG
    zero_row = 8 * np.array(SS)[sg_all]
    g0_all = rr[e0, all_t]
    g1_all = np.where(e0 == e1, zero_row, rr[e1, all_t])
    assert (g0_all >= 0).all() and (g1_all >= 0).all()
    for dcore in range(NCORES):
        sl = slice(dcore * OWN, (dcore + 1) * OWN)
        in_maps[dcore]["g0"] = g0_all[sl].astype(np.int32)
        in_maps[dcore]["g1"] = g1_all[sl].astype(np.int32)

    return in_maps, plan


def kernel(x, topk_e, topk_w, w1, w2):
    in_maps, plan = prepare_in_maps(x, topk_e, topk_w, w1, w2)
    nc = _build_program(plan)
    res = run_bass_kernel_spmd(nc, in_maps, list(range(NCORES)))
    out = np.concatenate([res.results[d]["yout"] for d in range(NCORES)], axis=0)
    return out.astype(np.float32)


# revision 13
# speedup vs baseline: 117416.0640x; 1.0434x over previous
"""Expert-parallel MoE MLP (top-2 of 8 experts) on 8 TRN2 NeuronCores.

Strategy (expert-parallel, per sharding hint):
  - core e holds expert e's weights (w1[e], w2[e], host-pre-transposed, bf16)
  - host dispatches tokens by expert id into a COMPACT layout: core e's
    xT holds exactly its routed tokens (ordered by owned-segment, then
    owner, then token id), padded only at the tail to a multiple of 128.
    This minimizes matmul columns (vs block-padded dispatch).
  - core e computes y = [silu(x_e @ w1[e]^T) * c_e] @ w2[e]^T over
    512-column chunks (bf16 matmuls, fp32 accumulate).
  - mm2 output m-tiles are indirect-scattered into `sendbuf`, which holds
    the AllToAll wire layout: NSEG slot-segments, stored in REVERSE
    segment order (plus a leading trash region for pad rows) so that each
    scatter's AP byte-extent is a prefix that never overlaps the A2A read
    slices of earlier-fired segments (avoids false WAR serialization).
  - NSEG chunked AllToAlls fire as soon as the last m-tile holding each
    segment's tokens has been scattered; segment s holds the partial rows
    of owned-token m-tile s on every owner, so the owner-side combine
    (2 indirect row-gathers + add) runs right after each A2A lands,
    overlapped with remaining compute and later A2As.
  - 16 dummy matmuls at t=0 warm the PE HAM clock gate during the loads.
"""

import sys

sys.path.insert(0, "/opt/trn_rl_repo")

import numpy as np
import ml_dtypes

import concourse.bass as bass
import concourse.tile as tile
from concourse import bacc, mybir
from concourse.bass_utils import run_bass_kernel_spmd

S, DM, DF, E, TOPK = 4096, 1024, 2048, 8, 2
NCORES = 8
P = 128
OWN = S // NCORES  # tokens per owner core
OWNSEG = 128  # owned tokens per A2A segment (= one combine m-tile)
NSEG = OWN // OWNSEG

_PROGRAM_CACHE: dict = {}


def _chunks_of(ntok: int) -> list[tuple[int, int]]:
    """Split ntok into (start, size) chunks, each a multiple of 128, <= 512."""
    out, pos = [], 0
    while pos < ntok:
        sz = min(512, ntok - pos)
        out.append((pos, sz))
        pos += sz
    return out


def _emit(nc, tc, ctx, plan: dict):
    dt = mybir.dt
    ntok = plan["ntok"]
    SS = plan["SS"]  # per-seg slot-block size
    FI = plan["FI"]  # fire A2A-s after scatter of m-tile FI[s]
    nmt = ntok // P

    SCOPS = plan["SCOPS"]  # per-m-tile scatter ops: list of (tm, seg)
    n_ops = len(SCOPS)

    xT = nc.dram_tensor("xT", [DM, ntok], dt.bfloat16, kind="ExternalInput").ap()
    w1t = nc.dram_tensor("w1t", [DM, DF], dt.bfloat16, kind="ExternalInput").ap()
    w2t = nc.dram_tensor("w2t", [DF, DM], dt.bfloat16, kind="ExternalInput").ap()
    cv = nc.dram_tensor("cv", [ntok], dt.float32, kind="ExternalInput").ap()
    scat = nc.dram_tensor("scat", [P * n_ops], dt.int32, kind="ExternalInput").ap()
    g0 = nc.dram_tensor("g0", [OWN], dt.int32, kind="ExternalInput").ap()
    g1 = nc.dram_tensor("g1", [OWN], dt.int32, kind="ExternalInput").ap()
    yout = nc.dram_tensor("yout", [OWN, DM], dt.float32, kind="ExternalOutput").ap()
    # one sendbuf per segment: indirect-scatter writes are tracked
    # conservatively (whole tensor), so per-seg tensors keep seg-s scatters
    # independent of other segments' in-flight AllToAll reads. Last P rows
    # of each are a trash region for pad tokens.
    sb = [
        nc.dram_tensor(f"send{s}", [8 * SS[s] + P, DM], dt.bfloat16).ap()
        for s in range(NSEG)
    ]
    recv = [
        nc.dram_tensor(f"recv{s}", [8 * SS[s] + 1, DM], dt.bfloat16).ap()
        for s in range(NSEG)
    ]

    dsend = nc.dram_tensor("dsend", [NCORES, 64], dt.bfloat16).ap()
    drecv = nc.dram_tensor("drecv", [NCORES, 64], dt.bfloat16).ap()

    wpool = ctx.enter_context(tc.tile_pool(name="w", bufs=1))
    hpool = ctx.enter_context(tc.tile_pool(name="h", bufs=34))
    ypool = ctx.enter_context(tc.tile_pool(name="y", bufs=10))
    gpool = ctx.enter_context(tc.tile_pool(name="g", bufs=2))
    phpool = ctx.enter_context(tc.tile_pool(name="ph", bufs=2, space="PSUM"))
    pypool = ctx.enter_context(tc.tile_pool(name="py", bufs=4, space="PSUM"))
    pwpool = ctx.enter_context(tc.tile_pool(name="pw", bufs=1, space="PSUM"))

    # ---- dummy collective: absorb cross-core start skew + ncfw first-call
    # cost while loads/compute proceed (its data is never consumed)
    nc.gpsimd.collective_compute(
        "AllToAll",
        mybir.AluOpType.bypass,
        replica_groups=[list(range(NCORES))],
        ins=[dsend[:, :]],
        outs=[drecv[:, :]],
    )

    # ---- PE warmup: release the HAM clock gate while DMAs load ----
    warm = wpool.tile([P, 512], dt.bfloat16, tag="warm")
    nc.vector.memset(warm[:], 0.0)
    pw = pwpool.tile([P, 512], dt.float32, tag="pw")
    for _ in range(16):
        nc.tensor.matmul(pw[:], lhsT=warm[:, 0:P], rhs=warm[:], start=True, stop=True)

    # ---- loads: few big DMAs, ordered by first use. mm1's i-loop needs all
    # k-tiles of w1 for one 128-wide f-slice, so load w1 f-major.
    w1sb = wpool.tile([P, DM // P, DF], dt.bfloat16, tag="w1sb")
    w1r = w1t.rearrange("(o p) f -> p o f", p=P)
    xsb = wpool.tile([P, DM // P, ntok], dt.bfloat16, tag="xsb")
    xr = xT.rearrange("(o p) t -> p o t", p=P)
    nc.sync.dma_start(w1sb[:, :, 0:512], w1r[:, :, 0:512])
    nc.sync.dma_start(xsb[:, :, 0 : min(512, ntok)], xr[:, :, 0 : min(512, ntok)])
    for fs in range(512, DF, 512):
        nc.sync.dma_start(w1sb[:, :, fs : fs + 512], w1r[:, :, fs : fs + 512])
    if ntok > 512:
        nc.sync.dma_start(xsb[:, :, 512:ntok], xr[:, :, 512:ntok])
    w2sb = wpool.tile([P, DF // P, DM], dt.bfloat16, tag="w2sb")
    w2r = w2t.rearrange("(o p) d -> p o d", p=P)
    nc.sync.dma_start(w2sb[:, 0:8, :], w2r[:, 0:8, :])
    nc.sync.dma_start(w2sb[:, 8:16, :], w2r[:, 8:16, :])
    csb = wpool.tile([P, nmt], dt.float32, tag="csb")
    nc.sync.dma_start(csb[:], cv.rearrange("(t p) -> p t", p=P))
    scatsb = wpool.tile([P, n_ops], dt.int32, tag="scatsb")
    nc.sync.dma_start(scatsb[:], scat.rearrange("(t p) -> p t", p=P))
    g0sb = wpool.tile([P, OWN // P], dt.int32, tag="g0sb")
    nc.sync.dma_start(g0sb[:], g0.rearrange("(t p) -> p t", p=P))
    g1sb = wpool.tile([P, OWN // P], dt.int32, tag="g1sb")
    nc.sync.dma_start(g1sb[:], g1.rearrange("(t p) -> p t", p=P))
    zrow = wpool.tile([1, DM], dt.bfloat16, tag="zrow")
    nc.vector.memset(zrow[:], 0.0)
    for s in range(NSEG):
        nc.sync.dma_start(recv[s][8 * SS[s] : 8 * SS[s] + 1, :], zrow[:])

    # ---- combine for owned m-tile s: gather partial 0, gather-accumulate
    # partial 1 via the DMA CCE adder, write out with a casting SWDGE DMA
    def combine(s):
        ga = gpool.tile([P, DM], dt.bfloat16, tag="ga")
        nc.gpsimd.indirect_dma_start(
            out=ga[:],
            out_offset=None,
            in_=recv[s][:],
            in_offset=bass.IndirectOffsetOnAxis(ap=g0sb[:, s : s + 1], axis=0),
        )
        nc.gpsimd.indirect_dma_start(
            out=ga[:],
            out_offset=None,
            in_=recv[s][:],
            in_offset=bass.IndirectOffsetOnAxis(ap=g1sb[:, s : s + 1], axis=0),
            compute_op=mybir.AluOpType.add,
        )
        nc.gpsimd.dma_start(yout[s * P : (s + 1) * P, :], ga[:])

    a2a_next = 0

    def maybe_fire(tm):
        nonlocal a2a_next
        while a2a_next < NSEG and FI[a2a_next] == tm:
            s = a2a_next
            nc.gpsimd.collective_compute(
                "AllToAll",
                mybir.AluOpType.bypass,
                replica_groups=[list(range(NCORES))],
                ins=[sb[s][0 : 8 * SS[s], :]],
                outs=[recv[s][0 : 8 * SS[s], :]],
            )
            a2a_next += 1
            combine(s)

    # ---- expert MLP over 512-column chunks of the compact token axis ----
    for c0, csz in _chunks_of(ntok):
        hs = []
        for i in range(DF // P):  # f-tiles: H[f] = silu(w1 . x)
            ph = phpool.tile([P, csz], dt.float32, tag="ph")
            for k in range(DM // P):
                nc.tensor.matmul(
                    ph[:],
                    lhsT=w1sb[:, k, i * P : (i + 1) * P],
                    rhs=xsb[:, k, c0 : c0 + csz],
                    start=(k == 0),
                    stop=(k == DM // P - 1),
                )
            h_i = hpool.tile([P, csz], dt.bfloat16, tag="h")
            nc.scalar.activation(h_i[:], ph[:], mybir.ActivationFunctionType.Silu)
            hs.append(h_i)
        for tml in range(csz // P):
            tm = c0 // P + tml
            py0 = pypool.tile([P, 512], dt.float32, tag="py")
            py1 = pypool.tile([P, 512], dt.float32, tag="py")
            for f in range(DF // P):
                lhs = hs[f][:, tml * P : (tml + 1) * P]
                nc.tensor.matmul(
                    py0[:], lhsT=lhs, rhs=w2sb[:, f, 0:512],
                    start=(f == 0), stop=(f == DF // P - 1),
                )
                nc.tensor.matmul(
                    py1[:], lhsT=lhs, rhs=w2sb[:, f, 512:1024],
                    start=(f == 0), stop=(f == DF // P - 1),
                )
            y_sb = ypool.tile([P, DM], dt.bfloat16, tag="y")
            nc.vector.tensor_scalar_mul(y_sb[:, 0:512], py0[:], csb[:, tm : tm + 1])
            nc.vector.tensor_scalar_mul(
                y_sb[:, 512:1024], py1[:], csb[:, tm : tm + 1]
            )
            for oi, (otm, oseg) in enumerate(SCOPS):
                if otm != tm:
                    continue
                nc.gpsimd.indirect_dma_start(
                    out=sb[oseg][:, :],
                    out_offset=bass.IndirectOffsetOnAxis(
                        ap=scatsb[:, oi : oi + 1], axis=0
                    ),
                    in_=y_sb[:],
                    in_offset=None,
                    bounds_check=8 * SS[oseg] + P - 1,
                    oob_is_err=False,
                )
            maybe_fire(tm)
    assert a2a_next == NSEG, (a2a_next, FI, nmt)


def _build_program(plan: dict):
    key = ("v4", plan["ntok"], tuple(plan["SS"]), tuple(plan["FI"]), tuple(plan["SCOPS"]))
    if key in _PROGRAM_CACHE:
        return _PROGRAM_CACHE[key]
    from contextlib import ExitStack

    nc = bacc.Bacc(
        "TRN2",
        target_bir_lowering=False,
        debug=False,
        enable_asserts=True,
        num_devices=NCORES,
    )
    with tile.TileContext(nc) as tc:
        with ExitStack() as ctx:
            _emit(nc, tc, ctx, plan)
    nc.compile()
    _PROGRAM_CACHE[key] = nc
    return nc


def prepare_in_maps(x, topk_e, topk_w, w1, w2):
    """Host-side routing/dispatch. Returns (in_maps, plan)."""
    bf16 = ml_dtypes.bfloat16
    x = np.asarray(x)
    topk_e = np.asarray(topk_e)
    topk_w = np.asarray(topk_w)
    w1 = np.asarray(w1)
    w2 = np.asarray(w2)

    # per-token combine weight for each expert
    c = np.zeros((S, E), dtype=np.float32)
    np.add.at(c, (np.arange(S)[:, None], topk_e), topk_w.astype(np.float32))

    toks = [np.nonzero((topk_e == e).any(axis=1))[0] for e in range(E)]
    # group counts per (expert, owner, seg)
    cnt = np.zeros((E, NCORES, NSEG), dtype=np.int64)
    for e in range(E):
        d = toks[e] // OWN
        sg = (toks[e] % OWN) // OWNSEG
        np.add.at(cnt[e], (d, sg), 1)
    SS = [int(v) for v in cnt.max(axis=(0, 1))]
    seg_tot = cnt.sum(axis=1)  # [E, NSEG] tokens per (expert, seg)
    cum = np.concatenate(
        [np.zeros((E, 1), np.int64), np.cumsum(seg_tot, axis=1)], axis=1
    )  # [E, NSEG+1]
    n_e = cum[:, -1]
    ntok = int(-(-n_e.max() // P) * P)
    nmt = ntok // P
    FI = [min(int(-(-cum[:, s + 1].max() // P)) - 1, nmt - 1) for s in range(NSEG)]
    # scatter ops: for each m-tile, one op per segment present on ANY core
    # (pads ride on the m-tile's first op, into the trash region)
    segs_of_tile = [set() for _ in range(nmt)]
    for tm in range(nmt):
        for e in range(E):
            lo = tm * P
            hi = min((tm + 1) * P, int(n_e[e]))
            if lo >= hi:
                continue
            s_lo = int(np.searchsorted(cum[e], lo, side="right")) - 1
            s_hi = int(np.searchsorted(cum[e], hi - 1, side="right")) - 1
            for s in range(s_lo, s_hi + 1):
                segs_of_tile[tm].add(s)
        if not segs_of_tile[tm]:
            segs_of_tile[tm].add(NSEG - 1)
    SCOPS = [(tm, s) for tm in range(nmt) for s in sorted(segs_of_tile[tm])]
    plan = {"ntok": ntok, "SS": SS, "FI": FI, "SCOPS": SCOPS}

    BIG = np.int32(1 << 20)  # > bounds_check -> lane silently skipped
    # per-core compact dispatch + recv-row map
    rr = np.full((E, S), -1, dtype=np.int64)  # (expert, token) -> row in recv[seg]
    in_maps = []
    for e in range(E):
        te = toks[e]
        d = te // OWN
        sg = (te % OWN) // OWNSEG
        order = np.lexsort((te, d, sg))  # by (seg, owner, token)
        te_o, d_o, s_o = te[order], d[order], sg[order]
        gid = s_o * NCORES + d_o  # non-decreasing in compact order
        rank = np.arange(len(te_o)) - np.searchsorted(gid, gid, side="left")
        ss_o = np.array(SS)[s_o]
        send_row = d_o * ss_o + rank  # row within sb[seg]
        rr[e, te_o] = e * ss_o + rank

        ne = len(te_o)
        xT_e = np.zeros((DM, ntok), dtype=bf16)
        xT_e[:, :ne] = x[te_o].T.astype(bf16)
        cv_e = np.zeros(ntok, dtype=np.float32)
        cv_e[:ne] = c[te_o, e]
        # per-op scatter index columns
        scat_e = np.full((len(SCOPS), P), BIG, dtype=np.int32)
        for oi, (tm, s_op) in enumerate(SCOPS):
            first_op = s_op == min(segs_of_tile[tm])
            for lane in range(P):
                pos = tm * P + lane
                if pos < ne:
                    if s_o[pos] == s_op:
                        scat_e[oi, lane] = send_row[pos]
                elif first_op:
                    scat_e[oi, lane] = 8 * SS[s_op] + lane  # trash
        in_maps.append(
            {
                "xT": xT_e,
                "cv": cv_e,
                "scat": scat_e.reshape(-1),
                "w1t": np.ascontiguousarray(w1[e].T).astype(bf16),
                "w2t": np.ascontiguousarray(w2[e].T).astype(bf16),
            }
        )

    # owner-side gather indices
    all_t = np.arange(S)
    e0 = topk_e[:, 0]
    e1 = topk_e[:, 1]
    sg_all = (all_t % OWN) // OWNSEG
    zero_row = 8 * np.array(SS)[sg_all]
    g0_all = rr[e0, all_t]
    g1_all = np.where(e0 == e1, zero_row, rr[e1, all_t])
    assert (g0_all >= 0).all() and (g1_all >= 0).all()
    for dcore in range(NCORES):
        sl = slice(dcore * OWN, (dcore + 1) * OWN)
        in_maps[dcore]["g0"] = g0_all[sl].astype(np.int32)
        in_maps[dcore]["g1"] = g1_all[sl].astype(np.int32)

    return in_maps, plan


def kernel(x, topk_e, topk_w, w1, w2):
    in_maps, plan = prepare_in_maps(x, topk_e, topk_w, w1, w2)
    nc = _build_program(plan)
    res = run_bass_kernel_spmd(nc, in_maps, list(range(NCORES)))
    out = np.concatenate([res.results[d]["yout"] for d in range(NCORES)], axis=0)
    return out.astype(np.float32)
